# revision 14
# baseline (speedup 1.0000x reference)
"""Trainium2 Bass kernel for ContractLevelAttention (segment softmax-pooling).

Computes, for x:[N,D], sorted batch:[N] (graph ids in [0,B)), MLP weights:
    scores = tanh(x @ W1 + b1) @ W2 + b2              # [N]
    w      = segment_softmax(scores, batch)           # per-graph softmax
    out    = segment_sum(x * w[:, None], batch)       # [B, D]

Key facts exploited:
  * softmax is shift invariant and |scores| <= 1 + 128*max|W2| + |b2| ~ 11.5
    (tanh output bounded), so exp() never overflows in fp32 and the
    segment-max subtraction of the reference can be dropped entirely.
  * out[g] = (sum_i e_i x_i) / (sum_i e_i) over i in graph g, so the
    normalization happens once at the end -- both sums are plain
    segment-sums, done as one-hot matmuls on the PE.
  * the pooling PE path runs in bf16 (1 cyc/row); the MLP path runs on a
    separate fp8-e4m3 pre-transposed copy of x with a DoubleRow matmul
    (2 contraction rows/cycle) -- softmax weights tolerate the fp8 score
    quantization (~1e-2 rel out err vs the 2e-2 gate).
  * x ships from host in ONE fused per-chunk HBM stream: [bf16 SBUF-layout
    chunk | fp8 transposed chunk] so each chunk is a single 12.4KB/partition
    contiguous DMA (line-rate HBM, one completion semaphore per chunk).
  * the one-hot build keeps every non-scalar operand 2-byte/SBUF so the DVE
    runs its 4x perf mode.

Sharding: graph-level data parallel over 8 cores (batch is sorted, so each
core's nodes are one contiguous slice, zero-padded to a fixed capacity).
"""

import numpy as np
from contextlib import ExitStack

N_FULL = 524288
D = 256
H = 128
B_FULL = 2048
NCORES = 8
B_LOC = B_FULL // NCORES      # 256 graphs per core
GCH = 128                     # graphs per PSUM accumulator chunk
NCH = B_LOC // GCH            # accumulator chunks per core
PAD_SENTINEL = 3.0 * B_LOC    # batch_rel value for padding rows (never matches)
CHT = 16                      # 128-node tiles per x DMA chunk
STT = 4                       # tiles per compute supertile (PSUM: 1 bank)
DA = D + 2                    # x cols + ones col (denominator) + pad col
XB = CHT * DA * 2             # bytes/partition of the bf16 x part of a chunk
TB = 2 * CHT * 128            # bytes/partition of the fp8 xT part of a chunk
MB = XB + TB                  # merged chunk bytes per partition

_prog_cache = {}

BUFS = {"xbp": 4, "ttp": 3, "oep": 6, "ep": 3}


def _build_program(C, bnds, nt_real, repeat=1, ablate=""):
    """Build the per-core SPMD program. C = padded node capacity (multiple of
    128*CHT). bnds = tuple of (first_tile, last_tile) per graph chunk, shared
    across cores (min/max over cores). nt_real = number of tiles containing
    any real (non-padding) node. repeat>1 wraps the body in an on-device
    loop (for timing)."""
    import concourse.bass as bass
    from concourse import bacc, mybir
    import concourse.tile as tile

    f32 = mybir.dt.float32
    bf16 = mybir.dt.bfloat16
    f8 = mybir.dt.float8e4
    u8 = mybir.dt.uint8
    AFT = mybir.ActivationFunctionType
    ALU = mybir.AluOpType
    DR = mybir.MatmulPerfMode.DoubleRow
    T = C // 128
    NCHK = T // CHT

    nc = bacc.Bacc(
        "TRN2",
        target_bir_lowering=False,
        debug=False,
        enable_asserts=False,
        num_devices=NCORES,
    )
    # fused per-chunk stream: [bf16 x in SBUF chunk layout | fp8 xT chunk]
    xm_d = nc.dram_tensor("xm", [128, NCHK * MB], u8, kind="ExternalInput").ap()
    brel_d = nc.dram_tensor("brel", [128, T], f32, kind="ExternalInput").ap()
    w1_d = nc.dram_tensor("w1", [2, 128, H], f32, kind="ExternalInput").ap()
    b1_d = nc.dram_tensor("b1", [H, 1], f32, kind="ExternalInput").ap()
    w2_d = nc.dram_tensor("w2", [H, 1], f32, kind="ExternalInput").ap()
    b2_d = nc.dram_tensor("b2", [128, 1], f32, kind="ExternalInput").ap()
    iota_d = nc.dram_tensor("iota", [128, B_LOC], f32, kind="ExternalInput").ap()
    out_d = nc.dram_tensor("out", [B_LOC, D], f32, kind="ExternalOutput").ap()

    first = {c: bnds[c][0] for c in range(NCH)}
    last = {c: bnds[c][1] for c in range(NCH)}

    with tile.TileContext(nc) as tc, ExitStack() as ctx:
        const = ctx.enter_context(tc.tile_pool(name="const", bufs=1))
        xbp = ctx.enter_context(tc.tile_pool(name="xbp", bufs=BUFS["xbp"]))
        ttp = ctx.enter_context(tc.tile_pool(name="ttp", bufs=BUFS["ttp"]))
        ep = ctx.enter_context(tc.tile_pool(name="ep", bufs=BUFS["ep"]))
        oep = ctx.enter_context(tc.tile_pool(name="oep", bufs=BUFS["oep"]))
        outp = ctx.enter_context(tc.tile_pool(name="outp", bufs=2))
        smallp = ctx.enter_context(tc.tile_pool(name="smallp", bufs=4))
        ps_u = ctx.enter_context(tc.tile_pool(name="ps_u", bufs=2, space="PSUM"))
        ps_s = ctx.enter_context(tc.tile_pool(name="ps_s", bufs=2, space="PSUM"))
        ps_acc = ctx.enter_context(tc.tile_pool(name="ps_acc", bufs=2, space="PSUM"))

        # --- constants, loaded once (low-precision casts made on-chip) ---
        brel_s = const.tile([128, T], f32)
        nc.sync.dma_start(brel_s[:], brel_d[:])
        b1_s = const.tile([128, 1], f32)
        nc.sync.dma_start(b1_s[:], b1_d[:])
        b2_s = const.tile([128, 1], f32)
        nc.sync.dma_start(b2_s[:], b2_d[:])
        iotaf_s = const.tile([128, B_LOC], f32)
        nc.sync.dma_start(iotaf_s[:], iota_d[:])
        iota_s = const.tile([128, B_LOC], bf16)  # bf16: enables DVE 4x mode
        nc.vector.tensor_copy(iota_s[:], iotaf_s[:])
        w1f_s = const.tile([128, 256], f32)
        nc.sync.dma_start(w1f_s[:, 0:128], w1_d[0])
        nc.sync.dma_start(w1f_s[:, 128:256], w1_d[1])
        w1_8 = const.tile([128, 2, 128], f8)
        nc.vector.tensor_copy(
            w1_8[:, :, :], w1f_s[:, :].rearrange("p (b h) -> p b h", b=2)
        )
        w2f_s = const.tile([128, 1], f32)
        nc.sync.dma_start(w2f_s[:], w2_d[:])
        w2_s = const.tile([128, 1], bf16)
        nc.vector.tensor_copy(w2_s[:], w2f_s[:])

        score_on = ablate not in ("noscore", "dmaonly")
        pool_on = ablate not in ("nopool", "dmaonly")

        def body(_iv=None):
            acc = {}
            for t0 in range(0, T, CHT):
                ci = t0 // CHT
                # one fused line-rate DMA per chunk: bf16 x + fp8 xT
                xmb = xbp.tile([128, MB], u8, tag="xmb")
                nc.sync.dma_start(xmb[:, :], xm_d[:, ci * MB : (ci + 1) * MB])
                xcb = xmb[:, 0:XB].bitcast(bf16)          # [128, CHT*DA]
                xtc = (
                    xmb[:, XB:MB]
                    .bitcast(f8)
                    .rearrange("p (b n) -> p b n", b=2)   # [128, 2, CHT*128]
                )
                nt = min(CHT, max(0, nt_real - t0))  # tiles with real nodes
                s_ps = ps_s.tile([128, CHT], f32, tag="sps")
                for st in range(t0, t0 + nt, STT):
                    if not score_on:
                        break
                    soff = (st - t0) * 128
                    # --- MLP: U^T[h, n], fp8 DoubleRow (both d-blocks in one
                    # matmul at 2 rows/cycle) ---
                    u_ps = ps_u.tile([128, STT * 128], f32, tag="ups")
                    nc.tensor.matmul(
                        u_ps[:],
                        w1_8[:, :, :],
                        xtc[:, :, soff : soff + STT * 128],
                        start=True,
                        stop=True,
                        perf_mode=DR,
                    )
                    tt_s = ttp.tile([128, STT * 128], bf16, tag="tts")
                    nc.scalar.activation(
                        tt_s[:], u_ps[:], AFT.Tanh, bias=b1_s[:]
                    )
                    for j in range(STT):
                        col = (st - t0) + j
                        nc.tensor.matmul(
                            s_ps[:, col : col + 1],
                            tt_s[:, j * 128 : (j + 1) * 128],
                            w2_s[:],
                            start=True,
                            stop=True,
                        )
                # --- e = exp(s + b2) for the whole chunk ---
                e_s = ep.tile([128, CHT], f32, tag="es")
                if score_on and nt > 0:
                    nc.scalar.activation(
                        e_s[:, 0:nt], s_ps[:, 0:nt], AFT.Exp, bias=b2_s[:]
                    )
                elif not score_on:
                    nc.vector.memset(e_s[:], 1.0)
                # --- pooling matmuls ---
                for j in range(CHT):
                    t = t0 + j
                    xb_t = xcb[:, j * DA : j * DA + DA]
                    for c in range(NCH):
                        if not pool_on:
                            continue
                        if not (first[c] <= t <= last[c]):
                            continue
                        if t == first[c]:
                            acc_t = ps_acc.tile([GCH, DA], f32, tag="acc")
                            acc[c] = acc_t
                        oe_s = oep.tile([128, GCH], bf16, tag="oes")
                        nc.vector.tensor_scalar(
                            oe_s[:],
                            iota_s[:, c * GCH : (c + 1) * GCH],
                            brel_s[:, t : t + 1],
                            e_s[:, j : j + 1],
                            op0=ALU.is_equal,
                            op1=ALU.mult,
                        )
                        nc.tensor.matmul(
                            acc[c][:, 0:DA],
                            oe_s[:],
                            xb_t[:],
                            start=(t == first[c]),
                            stop=(t == last[c]),
                        )
                        if t == last[c]:
                            den = smallp.tile([GCH, 1], f32, tag="den")
                            nc.vector.tensor_scalar_add(
                                den[:], acc[c][:, D : D + 1], 1e-30
                            )
                            rec = smallp.tile([GCH, 1], f32, tag="rec")
                            nc.vector.reciprocal(rec[:], den[:])
                            o_s = outp.tile([GCH, D], f32, tag="os")
                            nc.vector.tensor_scalar_mul(o_s[:], acc[c][:, 0:D], rec[:])
                            nc.sync.dma_start(out_d[c * GCH : (c + 1) * GCH, :], o_s[:])

        if repeat == 1:
            body()
        else:
            with tc.For_i(0, repeat, 1) as _i:
                body(_i)
    nc.compile()
    return nc


def _get_program(C, bnds, nt_real, repeat=1, ablate=""):
    key = (C, bnds, nt_real, repeat, ablate)
    if key not in _prog_cache:
        _prog_cache[key] = _build_program(C, bnds, nt_real, repeat, ablate)
    return _prog_cache[key]


def _prep_inputs(x, batch, W1, b1, W2, b2):
    """Host-side sharding: split nodes at graph boundaries, pad to fixed C,
    and pack the fused per-chunk HBM stream (pure layout prep + dtype casts,
    no arithmetic)."""
    import ml_dtypes

    x = np.ascontiguousarray(x, dtype=np.float32)
    batch = np.asarray(batch)
    W1 = np.ascontiguousarray(W1, dtype=np.float32)

    bounds = np.searchsorted(batch, np.arange(0, B_FULL + 1, B_LOC))  # [9]
    n_k = np.diff(bounds)
    cap = int(n_k.max())
    gran = 128 * CHT
    C = max(gran, ((cap + gran - 1) // gran) * gran)
    T = C // 128
    NCHK = T // CHT
    nt_real = (cap + 127) // 128

    # graph-chunk tile ranges, shared across cores (min/max over cores)
    bnds = []
    for c in range(NCH):
        los, his = [], []
        for k in range(NCORES):
            g0 = k * B_LOC + c * GCH
            g1 = g0 + GCH
            lo = int(np.searchsorted(batch, g0)) - int(bounds[k])
            hi = int(np.searchsorted(batch, g1)) - int(bounds[k])
            los.append(lo // 128)
            his.append((hi - 1) // 128 if hi > 0 else 0)
        ft = max(0, min(los))
        lt = min(T - 1, max(his))
        if c == NCH - 1:
            lt = nt_real - 1  # padding rows never match any graph
        bnds.append((ft, lt))
    bnds = tuple(bnds)

    shared = {
        "w1": W1.reshape(2, 128, H),
        "b1": np.ascontiguousarray(b1, dtype=np.float32).reshape(H, 1),
        "w2": np.ascontiguousarray(W2, dtype=np.float32).reshape(H, 1),
        "b2": np.full((128, 1), float(np.asarray(b2).reshape(-1)[0]), np.float32),
        "iota": np.broadcast_to(
            np.arange(B_LOC, dtype=np.float32), (128, B_LOC)
        ).copy(),
    }

    in_maps = []
    for k in range(NCORES):
        s, e = int(bounds[k]), int(bounds[k + 1])
        n = e - s
        # bf16 x (pooling path) in the exact SBUF chunk layout
        xk = np.zeros((C, DA), ml_dtypes.bfloat16)
        xk[:n, :D] = x[s:e].astype(ml_dtypes.bfloat16)
        xk[:, D] = 1.0  # denominator ones column (pad rows masked by one-hot)
        x_sb = xk.reshape(T, 128, DA).transpose(1, 0, 2)  # [128, T, DA]
        x_bytes = np.ascontiguousarray(x_sb).view(np.uint8).reshape(
            128, NCHK, CHT * DA * 2
        )
        # fp8 e4m3 pre-transposed copy (MLP path): [p, b, n] = x[n, b*128+p]
        xt8 = np.zeros((2, 128, C), ml_dtypes.float8_e4m3)
        xt8.reshape(D, C)[:, :n] = x[s:e].T.astype(ml_dtypes.float8_e4m3)
        xt_bytes = (
            xt8.reshape(2, 128, NCHK, CHT * 128)
            .transpose(1, 2, 0, 3)  # [128, NCHK, 2, CHT*128]
            .reshape(128, NCHK, TB)
            .view(np.uint8)
        )
        xm = np.concatenate([x_bytes, xt_bytes], axis=2).reshape(128, NCHK * MB)
        br = np.full((C,), PAD_SENTINEL, np.float32)
        br[:n] = batch[s:e].astype(np.float32) - k * B_LOC
        in_maps.append(
            {"xm": np.ascontiguousarray(xm),
             "brel": np.ascontiguousarray(br.reshape(T, 128).T), **shared}
        )
    return in_maps, C, bnds, nt_real


def kernel(x, batch, W1, b1, W2, b2):
    from concourse.bass_utils import run_bass_kernel_spmd

    in_maps, C, bnds, nt_real = _prep_inputs(x, batch, W1, b1, W2, b2)
    nc = _get_program(C, bnds, nt_real)
    res = run_bass_kernel_spmd(nc, in_maps, list(range(NCORES)))
    out = np.concatenate([res.results[k]["out"] for k in range(NCORES)], axis=0)
    return np.ascontiguousarray(out, dtype=np.float32)
